# revision 13
# baseline (speedup 1.0000x reference)
"""AttentionBlock (GroupNorm32 + 8-head global self-attention + proj + residual)
on 8 TRN2 NeuronCores, data-parallel over batch (B=8 -> 1 image per core).

v2 pipeline (per-core [C=512, N=1024], channels on partitions):
  GN: per-channel bn_stats -> group stats via selector matmul -> affine fold.
  QKV: q/k/v projections, PSUM evacuations split between ACT and DVE.
  Attention, mt-granular software pipeline per head pair:
    S^T MMs (K=64, row groups 0/64 issued adjacently for PE tile concurrency)
    exp split: even head on ACT (table exp), odd head on DVE via Schraudolph
      bit-trick (round(s*A+B) as int16, bitcast to bf16 ~ exp(s*scale))
    PV accumulates [v|1]^T @ P^T over mt; row 64 = softmax denominator.
  Normalize: denominators DMA'd to DRAM, reloaded transposed [16h, 64] so the
    DVE reciprocal runs on free-dim 64 (it is free-dim-cost dominated), then
    broadcast back via stride-0 partition DMA; in-place bf16 multiply.
  proj: accumulate proj_wT.T @ oT plus residual via identity-f32r matmul;
    per-tile evacuation alternating ACT/DVE, per-tile output DMA.
  PE warmup matmuls during the GN head phase keep the HAM clock at 2.4GHz.
"""
import numpy as np

C = 512
NH = 8
D = 64
N = 1024
GROUPS = 32
GS = C // GROUPS  # 16 channels per group
EPS = 1e-5
B = 8
NT = N // 512     # 2 n-tiles of 512
CT = C // 128     # 4 channel tiles
MT = N // 128     # 8 m-tiles (sequence on partitions)

SCALE = float(D) ** -0.5
LOG2E = 1.4426950408889634
# Schraudolph exp for bf16: bits = round(s*A + B); bitcast int16->bf16
SCH_A = SCALE * 128.0 * LOG2E
SCH_B = 127.0 * 128.0 - 5.6
N_WARM = 28       # PE warmup matmuls issued under the GN head phase

TRACE = False     # test.py flips this for profiling runs

_cache = {}


def _build():
    import concourse.bass as bass
    import concourse.bacc as bacc
    import concourse.tile as tile
    import concourse.mybir as mybir

    F32 = mybir.dt.float32
    F32R = mybir.dt.float32r
    BF16 = mybir.dt.bfloat16
    I16 = mybir.dt.int16
    AF = mybir.ActivationFunctionType
    ALU = mybir.AluOpType
    nc = bacc.Bacc("TRN2", target_bir_lowering=False, debug=False,
                   enable_asserts=False, num_devices=1)

    x_d = nc.dram_tensor("x", [C, N], F32R, kind="ExternalInput").ap()
    qkv_wT_d = nc.dram_tensor("qkv_wT", [C, 3 * C], BF16, kind="ExternalInput").ap()
    proj_wT_d = nc.dram_tensor("proj_wT", [C, C], BF16, kind="ExternalInput").ap()
    qk_bias_d = nc.dram_tensor("qk_bias", [2 * C, 1], F32, kind="ExternalInput").ap()
    gn_w_d = nc.dram_tensor("gn_w", [C, 1], F32, kind="ExternalInput").ap()
    gn_b_d = nc.dram_tensor("gn_b", [C, 1], F32, kind="ExternalInput").ap()
    proj_be_d = nc.dram_tensor("proj_be", [C, 1], F32, kind="ExternalInput").ap()
    sel_d = nc.dram_tensor("sel", [C, GROUPS], F32R, kind="ExternalInput").ap()
    expander_d = nc.dram_tensor("expander", [GROUPS, C], F32R, kind="ExternalInput").ap()
    ident_d = nc.dram_tensor("ident", [128, 128], F32R, kind="ExternalInput").ap()
    rs_dram = nc.dram_tensor("rs_scratch", [NH, N], F32, kind="Internal").ap()
    rs2_dram = nc.dram_tensor("rs2_scratch", [NH, N], BF16, kind="Internal").ap()
    out_d = nc.dram_tensor("out", [C, N], F32, kind="ExternalOutput").ap()

    with tile.TileContext(nc) as tc:
        with tc.tile_pool(name="const", bufs=1) as const, \
             tc.tile_pool(name="big", bufs=1) as big, \
             tc.tile_pool(name="pT_pool", bufs=4) as pT_pool, \
             tc.tile_pool(name="small", bufs=2) as small, \
             tc.tile_pool(name="norm", bufs=4) as norm, \
             tc.tile_pool(name="ps512", bufs=2, space="PSUM") as ps512, \
             tc.tile_pool(name="pspv", bufs=2, space="PSUM") as pspv:

            # ---- input x first (GN head phase gates everything) ----
            x_sb = big.tile([128, CT, N], F32R)
            for ci in range(CT):
                nc.sync.dma_start(out=x_sb[:, ci, :],
                                  in_=x_d.rearrange("(t p) n -> p t n", p=128)[:, ci, :])

            # ---- PE warmup: keep HAM at full clock through the GN phase ----
            wsrc = const.tile([128, 512], BF16)
            nc.vector.memset(wsrc, 0.25)
            wps = ps512.tile([128, N], F32, tag="sT", name="warm")
            for i in range(N_WARM):
                nc.tensor.matmul(wps[:, 0:512], wsrc[:, 0:128], wsrc[:],
                                 start=True, stop=True)

            # ---- constants / weights ----
            qkv_wT = const.tile([128, CT, 3 * C], BF16)
            proj_wT = const.tile([128, CT, C], BF16)
            sel = const.tile([128, CT, GROUPS], F32R)
            expander = const.tile([GROUPS, CT, 128], F32R)
            gn_w = const.tile([128, CT, 1], F32)
            gn_b = const.tile([128, CT, 1], F32)
            qk_bias = const.tile([128, 2 * CT, 1], F32)
            proj_be = const.tile([128, CT, 1], F32)
            ident = const.tile([128, 128], F32R)
            eps_t = const.tile([GROUPS, 1], F32)

            nc.sync.dma_start(out=qkv_wT, in_=qkv_wT_d.rearrange("(t p) o -> p t o", p=128))
            nc.sync.dma_start(out=proj_wT, in_=proj_wT_d.rearrange("(t p) o -> p t o", p=128))
            nc.sync.dma_start(out=sel, in_=sel_d.rearrange("(t p) g -> p t g", p=128))
            nc.sync.dma_start(out=expander, in_=expander_d.rearrange("g (t p) -> g t p", p=128))
            nc.sync.dma_start(out=gn_w, in_=gn_w_d.rearrange("(t p) o -> p t o", p=128))
            nc.sync.dma_start(out=gn_b, in_=gn_b_d.rearrange("(t p) o -> p t o", p=128))
            nc.sync.dma_start(out=qk_bias, in_=qk_bias_d.rearrange("(t p) o -> p t o", p=128))
            nc.sync.dma_start(out=proj_be, in_=proj_be_d.rearrange("(t p) o -> p t o", p=128))
            nc.sync.dma_start(out=ident, in_=ident_d)
            nc.vector.memset(eps_t, EPS)

            # ---- GroupNorm ----
            hn = big.tile([128, CT, N], BF16, tag="hnout")
            stat_rhs = norm.tile([128, CT, 2], F32R, bufs=1)
            for ci in range(CT):
                bstats = norm.tile([128, 2, 6], F32, tag="bstats")
                xv = x_sb[:, ci, :].rearrange("p (s n) -> p s n", s=2)
                for s in range(2):
                    nc.vector.bn_stats(out=bstats[:, s, :], in_=xv[:, s, :])
                mv = norm.tile([128, 2], F32, tag="mv")
                nc.vector.bn_aggr(out=mv, in_=bstats)
                # stat_rhs[:, ci, 0] = mean_c ; [:, ci, 1] = var_c + mean_c^2
                nc.vector.tensor_copy(out=stat_rhs[:, ci, 0:1], in_=mv[:, 0:1])
                nc.vector.tensor_tensor(out=stat_rhs[:, ci, 1:2], in0=mv[:, 0:1],
                                        in1=mv[:, 0:1], op=ALU.mult)
                nc.vector.tensor_tensor(out=stat_rhs[:, ci, 1:2], in0=stat_rhs[:, ci, 1:2],
                                        in1=mv[:, 1:2], op=ALU.add)
            grp_ps = pspv.tile([GROUPS, 2], F32, tag="pv")
            for ci in range(CT):
                nc.tensor.matmul(grp_ps[:], sel[:, ci, :], stat_rhs[:, ci, :],
                                 start=(ci == 0), stop=(ci == CT - 1))
            # group stats -> mean_g, rstd_g
            gmean = norm.tile([GROUPS, 1], F32, bufs=1)
            gvar = norm.tile([GROUPS, 1], F32, bufs=1)
            nc.vector.tensor_scalar(out=gmean, in0=grp_ps[:, 0:1], scalar1=1.0 / GS,
                                    scalar2=None, op0=ALU.mult)
            nc.vector.tensor_scalar(out=gvar, in0=grp_ps[:, 1:2], scalar1=1.0 / GS,
                                    scalar2=None, op0=ALU.mult)
            gm2 = norm.tile([GROUPS, 1], F32, bufs=1)
            nc.vector.tensor_tensor(out=gm2, in0=gmean, in1=gmean, op=ALU.mult)
            nc.vector.tensor_tensor(out=gvar, in0=gvar, in1=gm2, op=ALU.subtract)
            nc.scalar.activation(out=gvar, in_=gvar, func=AF.Sqrt, bias=eps_t, scale=1.0)
            nc.vector.reciprocal(out=gvar, in_=gvar)
            grp2 = norm.tile([GROUPS, 2], F32R, bufs=1)
            nc.vector.tensor_copy(out=grp2[:, 0:1], in_=gmean)
            nc.vector.tensor_copy(out=grp2[:, 1:2], in_=gvar)
            # expand to per-channel; fold gn affine: hn = x*A + Bb
            for ci in range(CT):
                exp_ps = pspv.tile([128, 2], F32, tag="pv")
                nc.tensor.matmul(exp_ps[:], expander[:, ci, :], grp2[:],
                                 start=True, stop=True)
                A = norm.tile([128, 1], F32, tag="A")
                Bb = norm.tile([128, 1], F32, tag="Bb")
                nc.vector.tensor_tensor(out=A, in0=exp_ps[:, 1:2], in1=gn_w[:, ci, :],
                                        op=ALU.mult)
                nc.vector.tensor_tensor(out=Bb, in0=exp_ps[:, 0:1], in1=A, op=ALU.mult)
                nc.vector.tensor_tensor(out=Bb, in0=gn_b[:, ci, :], in1=Bb, op=ALU.subtract)
                nc.vector.tensor_scalar(out=hn[:, ci, :], in0=x_sb[:, ci, :], scalar1=A,
                                        scalar2=Bb, op0=ALU.mult, op1=ALU.add)

            # ---- QKV projections (evacuations split ACT/DVE) ----
            q_sb = big.tile([128, CT, N], BF16)
            k_sb = big.tile([128, CT, N], BF16)
            for mt in range(2 * CT):  # 8 output tiles of 128 chans (q then k)
                dest = q_sb if mt < CT else k_sb
                ps = ps512.tile([128, N], F32, tag="sT", name=f"qk_{mt}")
                for kt in range(CT):
                    for nt in range(NT):
                        nc.tensor.matmul(ps[:, 512 * nt:512 * (nt + 1)],
                                         qkv_wT[:, kt, 128 * mt:128 * (mt + 1)],
                                         hn[:, kt, 512 * nt:512 * (nt + 1)],
                                         start=(kt == 0), stop=(kt == CT - 1))
                if mt % 2 == 0:
                    nc.scalar.add(out=dest[:, mt % CT, :], in_=ps[:],
                                  add=qk_bias[:, mt, :])
                else:
                    nc.vector.tensor_scalar(out=dest[:, mt % CT, :],
                                            in0=ps[:], scalar1=qk_bias[:, mt, :],
                                            scalar2=None, op0=ALU.add)
            # vT: [m, (mt, h, dd)] head-interleaved, col 64 = ones, col 65 = pad
            vT = big.tile([128, MT, NH, D + 2], BF16)
            nc.vector.memset(vT[:, :, :, D:D + 1], 1.0)
            nc.vector.memset(vT[:, :, :, D + 1:D + 2], 0.0)
            for mt in range(MT):
                ps = ps512.tile([128, N], F32, tag="sT", name=f"v_{mt}")
                for kt in range(CT):
                    nc.tensor.matmul(ps[:, 0:512], hn[:, kt, 128 * mt:128 * (mt + 1)],
                                     qkv_wT[:, kt, 2 * C:3 * C],
                                     start=(kt == 0), stop=(kt == CT - 1))
                if mt % 2 == 0:
                    nc.vector.tensor_copy(
                        out=vT[:, mt, :, 0:D],
                        in_=ps[:, 0:512].rearrange("p (h d) -> p h d", h=NH))
                else:
                    nc.scalar.copy(
                        out=vT[:, mt, :, 0:D],
                        in_=ps[:, 0:512].rearrange("p (h d) -> p h d", h=NH))

            # ---- attention: mt-granular pipeline per head pair ----
            oT = big.tile([128, CT, N], BF16)

            def pair(t):
                ps_o = [pspv.tile([D + 2, N], F32, tag="pv", name=f"pso_{t}_{hh}")
                        for hh in range(2)]
                for mt in range(MT):
                    ps_s = [ps512.tile([128, N], F32, tag="sT", name=f"ps_s_{t}_{mt}_{hh}")
                            for hh in range(2)]
                    # S^T: adjacent row-group-disjoint MMs (rows 0-63 / 64-127)
                    for nt in range(NT):
                        for hh in range(2):
                            qp = hh * 64
                            nc.tensor.matmul(
                                ps_s[hh][:, 512 * nt:512 * (nt + 1)],
                                k_sb[qp:qp + 64, t, 128 * mt:128 * (mt + 1)],
                                q_sb[qp:qp + 64, t, 512 * nt:512 * (nt + 1)],
                                start=True, stop=True,
                                tile_position=(qp, 0))
                    pt = [pT_pool.tile([128, N], BF16, tag="pT", bufs=4,
                                       name=f"pt_{t}_{mt}_{hh}") for hh in range(2)]
                    # exp split: ACT table-exp for head even, DVE Schraudolph odd
                    nc.scalar.activation(out=pt[0], in_=ps_s[0], func=AF.Exp,
                                         scale=SCALE)
                    nc.vector.tensor_scalar(out=pt[1][:].bitcast(I16), in0=ps_s[1],
                                            scalar1=SCH_A, scalar2=SCH_B,
                                            op0=ALU.mult, op1=ALU.add)
                    # PV accumulation
                    for hh in range(2):
                        for nt in range(NT):
                            nc.tensor.matmul(ps_o[hh][:, 512 * nt:512 * (nt + 1)],
                                             vT[:, mt, 2 * t + hh, :],
                                             pt[hh][:, 512 * nt:512 * (nt + 1)],
                                             start=(mt == 0), stop=(mt == MT - 1))
                # evacuate unnormalized o + denominator row, split engines
                nc.scalar.copy(out=oT[0:D, t, :], in_=ps_o[0][0:D, :])
                nc.vector.tensor_copy(out=oT[D:128, t, :], in_=ps_o[1][0:D, :])
                rs0 = small.tile([1, N], F32, tag="rs0", name=f"rs0_{t}")
                rs1 = small.tile([1, N], F32, tag="rs1", name=f"rs1_{t}")
                nc.scalar.copy(out=rs0, in_=ps_o[0][D:D + 1, :])
                nc.vector.tensor_copy(out=rs1, in_=ps_o[1][D:D + 1, :])
                nc.sync.dma_start(out=rs_dram[2 * t:2 * t + 1, :], in_=rs0)
                nc.sync.dma_start(out=rs_dram[2 * t + 1:2 * t + 2, :], in_=rs1)

            def normalize(h_lo, h_hi, tag):
                nh_ = h_hi - h_lo
                rsb = small.tile([16 * nh_, D], F32, tag=f"rsb{tag}", bufs=1,
                                 name=f"rsb_{tag}")
                src = rs_dram[h_lo:h_hi, :].rearrange("h (c f) -> (h c) f", f=D)
                nc.sync.dma_start(out=rsb, in_=src)
                rsbb = small.tile([16 * nh_, D], BF16, tag=f"rsbb{tag}", bufs=1,
                                  name=f"rsbb_{tag}")
                nc.vector.reciprocal(out=rsb, in_=rsb)
                nc.vector.tensor_copy(out=rsbb, in_=rsb)
                dst = rs2_dram[h_lo:h_hi, :].rearrange("h (c f) -> (h c) f", f=D)
                nc.sync.dma_start(out=dst, in_=rsbb)
                for h in range(h_lo, h_hi):
                    qt, qp = h // 2, (h % 2) * 64
                    bc = small.tile([128, N], BF16, tag="bc", name=f"bc_{h}")
                    srcap = rs2_dram[h:h + 1, :]
                    nc.gpsimd.dma_start(out=bc[qp:qp + 64, :], in_=bass.AP(
                        tensor=srcap.tensor, offset=srcap.offset,
                        ap=[[0, 64]] + list(srcap.ap[1:])))
                    nc.vector.tensor_tensor(out=oT[qp:qp + 64, qt, :],
                                            in0=oT[qp:qp + 64, qt, :],
                                            in1=bc[qp:qp + 64, :], op=ALU.mult)

            pair(0)
            pair(1)
            pair(2)
            normalize(0, 6, "a")   # heads 0-5 normalize while pair 3 runs
            pair(3)
            normalize(6, 8, "b")

            # ---- output projection + residual via identity matmul ----
            out_sb = big.tile([128, CT, N], F32, tag="hnout")
            for ot in range(CT):
                ps = ps512.tile([128, N], F32, tag="sT", name=f"proj_{ot}")
                for kt in range(CT):
                    for nt in range(NT):
                        nc.tensor.matmul(ps[:, 512 * nt:512 * (nt + 1)],
                                         proj_wT[:, kt, 128 * ot:128 * (ot + 1)],
                                         oT[:, kt, 512 * nt:512 * (nt + 1)],
                                         start=(kt == 0), stop=False)
                for nt in range(NT):
                    nc.tensor.matmul(ps[:, 512 * nt:512 * (nt + 1)],
                                     ident[:],
                                     x_sb[:, ot, 512 * nt:512 * (nt + 1)],
                                     start=False, stop=True)
                if ot % 2 == 0:
                    nc.vector.tensor_scalar(out=out_sb[:, ot, :], in0=ps[:],
                                            scalar1=proj_be[:, ot, :], scalar2=None,
                                            op0=ALU.add)
                else:
                    nc.scalar.add(out=out_sb[:, ot, :], in_=ps[:],
                                  add=proj_be[:, ot, :])
                nc.sync.dma_start(
                    out=out_d.rearrange("(t p) n -> p t n", p=128)[:, ot, :],
                    in_=out_sb[:, ot, :])

    nc.compile()
    return nc


def _host_prep(x, gn_w, gn_b, qkv_w, qkv_b, proj_w, proj_b):
    xf = np.ascontiguousarray(x.reshape(B, C, N), dtype=np.float32)
    import ml_dtypes
    qkv_wT = np.ascontiguousarray(qkv_w.T).astype(ml_dtypes.bfloat16)
    proj_wT = np.ascontiguousarray(proj_w.T).astype(ml_dtypes.bfloat16)
    proj_be = (proj_b + proj_w @ qkv_b[2 * C:]).astype(np.float32).reshape(C, 1)
    qk_bias = np.ascontiguousarray(qkv_b[:2 * C], dtype=np.float32).reshape(2 * C, 1)
    cid = np.arange(C)
    sel = (cid[:, None] // GS == np.arange(GROUPS)[None, :]).astype(np.float32)
    expander = np.ascontiguousarray(sel.T)
    shared = {
        "qkv_wT": qkv_wT, "proj_wT": proj_wT, "qk_bias": qk_bias,
        "gn_w": np.asarray(gn_w, np.float32).reshape(C, 1),
        "gn_b": np.asarray(gn_b, np.float32).reshape(C, 1),
        "proj_be": proj_be, "sel": sel, "expander": expander,
        "ident": np.eye(128, dtype=np.float32),
    }
    return [{**shared, "x": np.ascontiguousarray(xf[i])} for i in range(B)]


_PATCHED = {}


def _enable_ldw_opt():
    """walrus's LDWEIGHTS pipelining pass is off in this harness's driver cmd;
    re-enable it for this kernel's compiles (PE serializes LDW+MM otherwise)."""
    if _PATCHED:
        return
    from concourse import bass_utils
    orig = bass_utils.run_command

    def patched(argv, **kw):
        argv = ["--enable-ldw-opt=true" if a == "--enable-ldw-opt=false" else a
                for a in argv]
        return orig(argv, **kw)

    bass_utils.run_command = patched
    _PATCHED["on"] = True


def kernel(x, gn_w, gn_b, qkv_w, qkv_b, proj_w, proj_b):
    from concourse import bass_utils
    in_maps = _host_prep(np.asarray(x), np.asarray(gn_w), np.asarray(gn_b),
                         np.asarray(qkv_w), np.asarray(qkv_b),
                         np.asarray(proj_w), np.asarray(proj_b))
    key = "nc"
    if key not in _cache:
        _cache[key] = _build()
    res = bass_utils.run_bass_kernel_spmd(_cache[key], in_maps,
                                          core_ids=list(range(B)), trace=TRACE)
    _cache["last_result"] = res
    out = np.stack([res.results[i]["out"] for i in range(B)])
    return out.reshape(B, C, 32, 32).astype(np.float32)


# revision 15
# speedup vs baseline: 1.3431x; 1.3431x over previous
"""AttentionBlock (GroupNorm32 + 8-head global self-attention + proj + residual)
on 8 TRN2 NeuronCores, data-parallel over batch (B=8 -> 1 image per core).

v2 pipeline (per-core [C=512, N=1024], channels on partitions):
  GN: per-channel bn_stats -> group stats via selector matmul -> affine fold.
  QKV: q/k/v projections, PSUM evacuations split between ACT and DVE.
  Attention, mt-granular software pipeline per head pair:
    S^T MMs (K=64, row groups 0/64 issued adjacently for PE tile concurrency)
    exp split: even head on ACT (table exp), odd head on DVE via Schraudolph
      bit-trick (round(s*A+B) as int16, bitcast to bf16 ~ exp(s*scale))
    PV accumulates [v|1]^T @ P^T over mt; row 64 = softmax denominator.
  Normalize: denominators DMA'd to DRAM, reloaded transposed [16h, 64] so the
    DVE reciprocal runs on free-dim 64 (it is free-dim-cost dominated), then
    broadcast back via stride-0 partition DMA; in-place bf16 multiply.
  proj: accumulate proj_wT.T @ oT plus residual via identity-f32r matmul;
    per-tile evacuation alternating ACT/DVE, per-tile output DMA.
  PE warmup matmuls during the GN head phase keep the HAM clock at 2.4GHz.
"""
import numpy as np

C = 512
NH = 8
D = 64
N = 1024
GROUPS = 32
GS = C // GROUPS  # 16 channels per group
EPS = 1e-5
B = 8
NT = N // 512     # 2 n-tiles of 512
CT = C // 128     # 4 channel tiles
MT = N // 128     # 8 m-tiles (sequence on partitions)

SCALE = float(D) ** -0.5
LOG2E = 1.4426950408889634
# Schraudolph exp for bf16: bits = round(s*A + B); bitcast int16->bf16
SCH_A = SCALE * 128.0 * LOG2E
SCH_B = 127.0 * 128.0 - 5.6
N_WARM = 28       # PE warmup matmuls issued under the GN head phase

TRACE = False     # test.py flips this for profiling runs

_cache = {}


def _build():
    import concourse.bass as bass
    import concourse.bacc as bacc
    import concourse.tile as tile
    import concourse.mybir as mybir

    F32 = mybir.dt.float32
    F32R = mybir.dt.float32r
    BF16 = mybir.dt.bfloat16
    I16 = mybir.dt.int16
    AF = mybir.ActivationFunctionType
    ALU = mybir.AluOpType
    nc = bacc.Bacc("TRN2", target_bir_lowering=False, debug=False,
                   enable_asserts=False, num_devices=1)

    x_d = nc.dram_tensor("x", [C, N], F32R, kind="ExternalInput").ap()
    qkv_wT_d = nc.dram_tensor("qkv_wT", [C, 3 * C], BF16, kind="ExternalInput").ap()
    proj_wT_d = nc.dram_tensor("proj_wT", [C, C], BF16, kind="ExternalInput").ap()
    qk_bias_d = nc.dram_tensor("qk_bias", [2 * C, 1], F32, kind="ExternalInput").ap()
    gn_w_d = nc.dram_tensor("gn_w", [C, 1], F32, kind="ExternalInput").ap()
    gn_b_d = nc.dram_tensor("gn_b", [C, 1], F32, kind="ExternalInput").ap()
    proj_be_d = nc.dram_tensor("proj_be", [C, 1], F32, kind="ExternalInput").ap()
    sel_d = nc.dram_tensor("sel", [C, GROUPS], F32R, kind="ExternalInput").ap()
    expander_d = nc.dram_tensor("expander", [GROUPS, C], F32R, kind="ExternalInput").ap()
    ident_d = nc.dram_tensor("ident", [128, 128], F32R, kind="ExternalInput").ap()
    rs_dram = nc.dram_tensor("rs_scratch", [NH, N], F32, kind="Internal").ap()
    rs2_dram = nc.dram_tensor("rs2_scratch", [NH, N], BF16, kind="Internal").ap()
    out_d = nc.dram_tensor("out", [C, N], F32, kind="ExternalOutput").ap()

    with tile.TileContext(nc) as tc:
        with tc.tile_pool(name="const", bufs=1) as const, \
             tc.tile_pool(name="big", bufs=1) as big, \
             tc.tile_pool(name="pT_pool", bufs=4) as pT_pool, \
             tc.tile_pool(name="small", bufs=2) as small, \
             tc.tile_pool(name="norm", bufs=4) as norm, \
             tc.tile_pool(name="ps1", bufs=6, space="PSUM") as ps1, \
             tc.tile_pool(name="pspv", bufs=2, space="PSUM") as pspv:

            # ---- input x first (GN head phase gates everything) ----
            x_sb = big.tile([128, CT, N], F32R)
            for ci in range(CT):
                nc.sync.dma_start(out=x_sb[:, ci, :],
                                  in_=x_d.rearrange("(t p) n -> p t n", p=128)[:, ci, :])

            # ---- PE warmup: keep HAM at full clock through the GN phase ----
            wsrc = const.tile([128, 512], BF16)
            nc.vector.memset(wsrc, 0.25)
            wps = ps1.tile([128, 512], F32, tag="sT", name="warm")
            for i in range(N_WARM):
                nc.tensor.matmul(wps[:], wsrc[:, 0:128], wsrc[:],
                                 start=True, stop=True)

            # ---- constants / weights ----
            qkv_wT = const.tile([128, CT, 3 * C], BF16)
            proj_wT = const.tile([128, CT, C], BF16)
            sel = const.tile([128, CT, GROUPS], F32R)
            expander = const.tile([GROUPS, CT, 128], F32R)
            gn_w = const.tile([128, CT, 1], F32)
            gn_b = const.tile([128, CT, 1], F32)
            qk_bias = const.tile([128, 2 * CT, 1], F32)
            proj_be = const.tile([128, CT, 1], F32)
            ident = const.tile([128, 128], F32R)
            eps_t = const.tile([GROUPS, 1], F32)

            nc.sync.dma_start(out=qkv_wT, in_=qkv_wT_d.rearrange("(t p) o -> p t o", p=128))
            nc.sync.dma_start(out=proj_wT, in_=proj_wT_d.rearrange("(t p) o -> p t o", p=128))
            nc.sync.dma_start(out=sel, in_=sel_d.rearrange("(t p) g -> p t g", p=128))
            nc.sync.dma_start(out=expander, in_=expander_d.rearrange("g (t p) -> g t p", p=128))
            nc.sync.dma_start(out=gn_w, in_=gn_w_d.rearrange("(t p) o -> p t o", p=128))
            nc.sync.dma_start(out=gn_b, in_=gn_b_d.rearrange("(t p) o -> p t o", p=128))
            nc.sync.dma_start(out=qk_bias, in_=qk_bias_d.rearrange("(t p) o -> p t o", p=128))
            nc.sync.dma_start(out=proj_be, in_=proj_be_d.rearrange("(t p) o -> p t o", p=128))
            nc.sync.dma_start(out=ident, in_=ident_d)
            nc.vector.memset(eps_t, EPS)

            # ---- GroupNorm ----
            hn = big.tile([128, CT, N], BF16, tag="hnout")
            stat_rhs = norm.tile([128, CT, 2], F32R, bufs=1)
            for ci in range(CT):
                bstats = norm.tile([128, 2, 6], F32, tag="bstats")
                xv = x_sb[:, ci, :].rearrange("p (s n) -> p s n", s=2)
                for s in range(2):
                    nc.vector.bn_stats(out=bstats[:, s, :], in_=xv[:, s, :])
                mv = norm.tile([128, 2], F32, tag="mv")
                nc.vector.bn_aggr(out=mv, in_=bstats)
                # stat_rhs[:, ci, 0] = mean_c ; [:, ci, 1] = var_c + mean_c^2
                nc.vector.tensor_copy(out=stat_rhs[:, ci, 0:1], in_=mv[:, 0:1])
                nc.vector.tensor_tensor(out=stat_rhs[:, ci, 1:2], in0=mv[:, 0:1],
                                        in1=mv[:, 0:1], op=ALU.mult)
                nc.vector.tensor_tensor(out=stat_rhs[:, ci, 1:2], in0=stat_rhs[:, ci, 1:2],
                                        in1=mv[:, 1:2], op=ALU.add)
            grp_ps = pspv.tile([GROUPS, 2], F32, tag="pv")
            for ci in range(CT):
                nc.tensor.matmul(grp_ps[:], sel[:, ci, :], stat_rhs[:, ci, :],
                                 start=(ci == 0), stop=(ci == CT - 1))
            # group stats -> mean_g, rstd_g
            gmean = norm.tile([GROUPS, 1], F32, bufs=1)
            gvar = norm.tile([GROUPS, 1], F32, bufs=1)
            nc.vector.tensor_scalar(out=gmean, in0=grp_ps[:, 0:1], scalar1=1.0 / GS,
                                    scalar2=None, op0=ALU.mult)
            nc.vector.tensor_scalar(out=gvar, in0=grp_ps[:, 1:2], scalar1=1.0 / GS,
                                    scalar2=None, op0=ALU.mult)
            gm2 = norm.tile([GROUPS, 1], F32, bufs=1)
            nc.vector.tensor_tensor(out=gm2, in0=gmean, in1=gmean, op=ALU.mult)
            nc.vector.tensor_tensor(out=gvar, in0=gvar, in1=gm2, op=ALU.subtract)
            nc.scalar.activation(out=gvar, in_=gvar, func=AF.Sqrt, bias=eps_t, scale=1.0)
            nc.vector.reciprocal(out=gvar, in_=gvar)
            grp2 = norm.tile([GROUPS, 2], F32R, bufs=1)
            nc.vector.tensor_copy(out=grp2[:, 0:1], in_=gmean)
            nc.vector.tensor_copy(out=grp2[:, 1:2], in_=gvar)
            # expand to per-channel; fold gn affine: hn = x*A + Bb
            for ci in range(CT):
                exp_ps = pspv.tile([128, 2], F32, tag="pv")
                nc.tensor.matmul(exp_ps[:], expander[:, ci, :], grp2[:],
                                 start=True, stop=True)
                A = norm.tile([128, 1], F32, tag="A")
                Bb = norm.tile([128, 1], F32, tag="Bb")
                nc.vector.tensor_tensor(out=A, in0=exp_ps[:, 1:2], in1=gn_w[:, ci, :],
                                        op=ALU.mult)
                nc.vector.tensor_tensor(out=Bb, in0=exp_ps[:, 0:1], in1=A, op=ALU.mult)
                nc.vector.tensor_tensor(out=Bb, in0=gn_b[:, ci, :], in1=Bb, op=ALU.subtract)
                nc.vector.tensor_scalar(out=hn[:, ci, :], in0=x_sb[:, ci, :], scalar1=A,
                                        scalar2=Bb, op0=ALU.mult, op1=ALU.add)

            # ---- QKV projections (evacuations split ACT/DVE) ----
            q_sb = big.tile([128, CT, N], BF16)
            k_sb = big.tile([128, CT, N], BF16)
            for mt in range(2 * CT):  # 8 output tiles of 128 chans (q then k)
                dest = q_sb if mt < CT else k_sb
                for nt in range(NT):
                    ps = ps1.tile([128, 512], F32, tag="sT", name=f"qk_{mt}_{nt}")
                    for kt in range(CT):
                        nc.tensor.matmul(ps[:],
                                         qkv_wT[:, kt, 128 * mt:128 * (mt + 1)],
                                         hn[:, kt, 512 * nt:512 * (nt + 1)],
                                         start=(kt == 0), stop=(kt == CT - 1))
                    dsl = dest[:, mt % CT, 512 * nt:512 * (nt + 1)]
                    if mt % 2 == 0:
                        nc.scalar.add(out=dsl, in_=ps[:], add=qk_bias[:, mt, :])
                    else:
                        nc.vector.tensor_scalar(out=dsl, in0=ps[:],
                                                scalar1=qk_bias[:, mt, :],
                                                scalar2=None, op0=ALU.add)
            # vT: [m, (mt, h, dd)] head-interleaved, col 64 = ones, col 65 = pad
            vT = big.tile([128, MT, NH, D + 2], BF16)
            nc.vector.memset(vT[:, :, :, D:D + 1], 1.0)
            nc.vector.memset(vT[:, :, :, D + 1:D + 2], 0.0)
            for mt in range(MT):
                ps = ps1.tile([128, 512], F32, tag="sT", name=f"v_{mt}")
                for kt in range(CT):
                    nc.tensor.matmul(ps[:], hn[:, kt, 128 * mt:128 * (mt + 1)],
                                     qkv_wT[:, kt, 2 * C:3 * C],
                                     start=(kt == 0), stop=(kt == CT - 1))
                if mt % 2 == 0:
                    nc.vector.tensor_copy(
                        out=vT[:, mt, :, 0:D],
                        in_=ps[:].rearrange("p (h d) -> p h d", h=NH))
                else:
                    nc.scalar.copy(
                        out=vT[:, mt, :, 0:D],
                        in_=ps[:].rearrange("p (h d) -> p h d", h=NH))

            # ---- attention: mt-granular pipeline per head pair ----
            oT = big.tile([128, CT, N], BF16)

            def pair(t):
                # two nt-half passes; each PV accumulator is one PSUM bank so
                # the sT ring stays 6 deep and S-MMs run ahead of exp
                for nh in range(NT):
                    ns = slice(512 * nh, 512 * (nh + 1))
                    ps_o = [pspv.tile([D + 2, 512], F32, tag="pv",
                                      name=f"pso_{t}_{nh}_{hh}") for hh in range(2)]
                    for mt in range(MT):
                        ps_s = [ps1.tile([128, 512], F32, tag="sT",
                                         name=f"ps_s_{t}_{nh}_{mt}_{hh}")
                                for hh in range(2)]
                        for hh in range(2):
                            qp = hh * 64
                            nc.tensor.matmul(
                                ps_s[hh][:],
                                k_sb[qp:qp + 64, t, 128 * mt:128 * (mt + 1)],
                                q_sb[qp:qp + 64, t, ns],
                                start=True, stop=True,
                                tile_position=(qp, 0))
                        pt = [pT_pool.tile([128, 512], BF16, tag="pT", bufs=6,
                                           name=f"pt_{t}_{nh}_{mt}_{hh}")
                              for hh in range(2)]
                        # exp split: ACT table-exp head even, DVE Schraudolph odd
                        nc.scalar.activation(out=pt[0], in_=ps_s[0], func=AF.Exp,
                                             scale=SCALE)
                        nc.vector.tensor_scalar(out=pt[1][:].bitcast(I16),
                                                in0=ps_s[1],
                                                scalar1=SCH_A, scalar2=SCH_B,
                                                op0=ALU.mult, op1=ALU.add)
                        for hh in range(2):
                            nc.tensor.matmul(ps_o[hh][:],
                                             vT[:, mt, 2 * t + hh, :],
                                             pt[hh][:],
                                             start=(mt == 0), stop=(mt == MT - 1))
                    # evacuate unnormalized o + denominator row, split engines
                    nc.scalar.copy(out=oT[0:D, t, ns], in_=ps_o[0][0:D, :])
                    nc.vector.tensor_copy(out=oT[D:128, t, ns], in_=ps_o[1][0:D, :])
                    rs0 = small.tile([1, 512], F32, tag="rs0", name=f"rs0_{t}_{nh}")
                    rs1 = small.tile([1, 512], F32, tag="rs1", name=f"rs1_{t}_{nh}")
                    nc.scalar.copy(out=rs0, in_=ps_o[0][D:D + 1, :])
                    nc.vector.tensor_copy(out=rs1, in_=ps_o[1][D:D + 1, :])
                    nc.sync.dma_start(out=rs_dram[2 * t:2 * t + 1, ns], in_=rs0)
                    nc.sync.dma_start(out=rs_dram[2 * t + 1:2 * t + 2, ns], in_=rs1)

            def normalize(h_lo, h_hi, tag):
                nh_ = h_hi - h_lo
                rsb = small.tile([16 * nh_, D], F32, tag=f"rsb{tag}", bufs=1,
                                 name=f"rsb_{tag}")
                src = rs_dram[h_lo:h_hi, :].rearrange("h (c f) -> (h c) f", f=D)
                nc.sync.dma_start(out=rsb, in_=src)
                rsbb = small.tile([16 * nh_, D], BF16, tag=f"rsbb{tag}", bufs=1,
                                  name=f"rsbb_{tag}")
                nc.vector.reciprocal(out=rsb, in_=rsb)
                nc.vector.tensor_copy(out=rsbb, in_=rsb)
                dst = rs2_dram[h_lo:h_hi, :].rearrange("h (c f) -> (h c) f", f=D)
                nc.sync.dma_start(out=dst, in_=rsbb)
                for h in range(h_lo, h_hi):
                    qt, qp = h // 2, (h % 2) * 64
                    bc = small.tile([128, N], BF16, tag="bc", name=f"bc_{h}")
                    srcap = rs2_dram[h:h + 1, :]
                    nc.gpsimd.dma_start(out=bc[qp:qp + 64, :], in_=bass.AP(
                        tensor=srcap.tensor, offset=srcap.offset,
                        ap=[[0, 64]] + list(srcap.ap[1:])))
                    nc.vector.tensor_tensor(out=oT[qp:qp + 64, qt, :],
                                            in0=oT[qp:qp + 64, qt, :],
                                            in1=bc[qp:qp + 64, :], op=ALU.mult)

            pair(0)
            pair(1)
            pair(2)
            normalize(0, 6, "a")   # heads 0-5 normalize while pair 3 runs
            pair(3)
            normalize(6, 8, "b")

            # ---- output projection + residual via identity matmul ----
            out_sb = big.tile([128, CT, N], F32, tag="hnout")
            for ot in range(CT):
                for nt in range(NT):
                    ns = slice(512 * nt, 512 * (nt + 1))
                    ps = ps1.tile([128, 512], F32, tag="sT", name=f"proj_{ot}_{nt}")
                    for kt in range(CT):
                        nc.tensor.matmul(ps[:],
                                         proj_wT[:, kt, 128 * ot:128 * (ot + 1)],
                                         oT[:, kt, ns],
                                         start=(kt == 0), stop=False)
                    nc.tensor.matmul(ps[:], ident[:], x_sb[:, ot, ns],
                                     start=False, stop=True)
                    if (2 * ot + nt) % 2 == 0:
                        nc.vector.tensor_scalar(out=out_sb[:, ot, ns], in0=ps[:],
                                                scalar1=proj_be[:, ot, :],
                                                scalar2=None, op0=ALU.add)
                    else:
                        nc.scalar.add(out=out_sb[:, ot, ns], in_=ps[:],
                                      add=proj_be[:, ot, :])
                nc.sync.dma_start(
                    out=out_d.rearrange("(t p) n -> p t n", p=128)[:, ot, :],
                    in_=out_sb[:, ot, :])

    nc.compile()
    return nc


def _host_prep(x, gn_w, gn_b, qkv_w, qkv_b, proj_w, proj_b):
    xf = np.ascontiguousarray(x.reshape(B, C, N), dtype=np.float32)
    import ml_dtypes
    qkv_wT = np.ascontiguousarray(qkv_w.T).astype(ml_dtypes.bfloat16)
    proj_wT = np.ascontiguousarray(proj_w.T).astype(ml_dtypes.bfloat16)
    proj_be = (proj_b + proj_w @ qkv_b[2 * C:]).astype(np.float32).reshape(C, 1)
    qk_bias = np.ascontiguousarray(qkv_b[:2 * C], dtype=np.float32).reshape(2 * C, 1)
    cid = np.arange(C)
    sel = (cid[:, None] // GS == np.arange(GROUPS)[None, :]).astype(np.float32)
    expander = np.ascontiguousarray(sel.T)
    shared = {
        "qkv_wT": qkv_wT, "proj_wT": proj_wT, "qk_bias": qk_bias,
        "gn_w": np.asarray(gn_w, np.float32).reshape(C, 1),
        "gn_b": np.asarray(gn_b, np.float32).reshape(C, 1),
        "proj_be": proj_be, "sel": sel, "expander": expander,
        "ident": np.eye(128, dtype=np.float32),
    }
    return [{**shared, "x": np.ascontiguousarray(xf[i])} for i in range(B)]


_PATCHED = {}


def _enable_ldw_opt():
    """walrus's LDWEIGHTS pipelining pass is off in this harness's driver cmd;
    re-enable it for this kernel's compiles (PE serializes LDW+MM otherwise)."""
    if _PATCHED:
        return
    from concourse import bass_utils
    orig = bass_utils.run_command

    def patched(argv, **kw):
        argv = ["--enable-ldw-opt=true" if a == "--enable-ldw-opt=false" else a
                for a in argv]
        return orig(argv, **kw)

    bass_utils.run_command = patched
    _PATCHED["on"] = True


def kernel(x, gn_w, gn_b, qkv_w, qkv_b, proj_w, proj_b):
    from concourse import bass_utils
    in_maps = _host_prep(np.asarray(x), np.asarray(gn_w), np.asarray(gn_b),
                         np.asarray(qkv_w), np.asarray(qkv_b),
                         np.asarray(proj_w), np.asarray(proj_b))
    key = "nc"
    if key not in _cache:
        _cache[key] = _build()
    res = bass_utils.run_bass_kernel_spmd(_cache[key], in_maps,
                                          core_ids=list(range(B)), trace=TRACE)
    _cache["last_result"] = res
    out = np.stack([res.results[i]["out"] for i in range(B)])
    return out.reshape(B, C, 32, 32).astype(np.float32)
